# revision 32
# baseline (speedup 1.0000x reference)
"""EdgeAttention GNN message passing on 8 Trainium2 NeuronCores.

Strategy (edge-parallel, receiver-packed, gather-free, fp8 senders):
  - Host: receivers are bin-packed into (core, block, window) bins --
    8 cores x 50 blocks x 4 windows of 32 receiver slots, each bin
    holding at most t=8 subtiles (1024 edge slots).  Receiver->slot
    assignment is a free permutation (host unpermutes the output), so
    edge counts balance almost perfectly and padding is ~2%.  The host
    materializes each edge slot's SENDER raw features (pure data
    movement) in fp8-e4m3 DoubleRow layout; edge features stay f16
    (the v-path error budget does not tolerate fp8; the k/q path does).
  - Device phase 1: q = lrelu(own_nodes @ Wq.T) via one fp8 DoubleRow
    matmul per 512 columns, kept in SBUF (f16).
  - Device phase 2, one batch == one (block, window) bin == 8 subtiles:
      kT_e = lrelu(Wk2 @r nodesT_e)   (fp8 DoubleRow PE + ACT Prelu)
      v    = lrelu(edgesT @ Wv.T)     (f16 PE per subtile; lrelu split
                                       ACT Prelu / DVE copy+max(t,.01t))
      S    = kT_e.T @ q_win           (f16 PE, 32 cols per subtile)
      Et   = exp(S / sqrt(d))         (ACT, one instr per batch)
      oh   = is_equal(rc, iota32)     (DVE); Et *= oh
      out_blk[W:W+32] += Et.T @ [v|1] (PE; col 128 = softmax denom;
                                       emitted one batch late to hide
                                       the exp/mask latency)
    Block epilogue: out = numer * reciprocal(denom) (DVE), DMA out.
  Softmax max-subtraction is skipped: logits are O(1) here (|l|<4) and
  exp(l)/sum(exp(l)) == exp(l-m)/sum(exp(l-m)) exactly in the reals.
"""

import sys

sys.path.insert(0, "/opt/trn_rl_repo")

import numpy as np
import ml_dtypes

N_CORES = 8
P = 128
WW = 32                              # receiver window width (slots per bin)
G = P // WW                          # windows per block
T_SUB = 8                            # subtiles per bin (bin edge cap T_SUB*P)

E4 = ml_dtypes.float8_e4m3
F16 = np.float16
BF16 = ml_dtypes.bfloat16


def _cfg_from_shapes(n_nodes, d_v, d_e, d_attn):
    assert d_v == 2 * P and d_e == P and d_attn == P
    npc = -(-n_nodes // (N_CORES * P)) * P + P      # one spare block of slack
    nb = npc // P
    t_b = G * T_SUB
    ntiles = nb * t_b
    return dict(
        NPC=npc, NB=nb, T_B=t_b, NTILES=ntiles, E_PAD=ntiles * P, DVC=d_v // P,
    )


def _pack(edge_index, n_nodes, cfg):
    """Assign receivers to (core, bin, slot); bins are (block, window).

    Returns core_of[r], bin_of[r], slot_of[r]."""
    import heapq

    NB = cfg["NB"]
    nbins = NB * G
    cap = T_SUB * P
    r = np.asarray(edge_index[1], dtype=np.int64)
    cnt = np.bincount(r, minlength=n_nodes)
    order = np.argsort(-cnt, kind="stable")

    core_of = np.full(n_nodes, -1, dtype=np.int64)
    bin_of = np.full(n_nodes, -1, dtype=np.int64)
    slot_of = np.full(n_nodes, -1, dtype=np.int64)

    # 1) balance edges across cores (receiver-count cap NPC slots)
    core_heap = [(0, c) for c in range(N_CORES)]
    heapq.heapify(core_heap)
    core_recv = [[] for _ in range(N_CORES)]
    max_recv = nbins * WW
    for node in order:
        while True:
            load, c = heapq.heappop(core_heap)
            if len(core_recv[c]) < max_recv:
                break
        core_recv[c].append(node)
        core_of[node] = c
        heapq.heappush(core_heap, (load + int(cnt[node]), c))

    # 2) per core: worst-fit-decreasing into bins (cap edges, 32 slots)
    for c in range(N_CORES):
        bins = [(-cap, b) for b in range(nbins)]
        heapq.heapify(bins)
        used = np.zeros(nbins, dtype=np.int64)
        nslot = np.zeros(nbins, dtype=np.int64)
        for node in core_recv[c]:          # already count-descending
            while True:
                negrem, b = heapq.heappop(bins)
                if nslot[b] < WW:
                    break
            rem = -negrem
            k = int(cnt[node])
            assert rem >= k, "bin packing overflow; raise T_SUB"
            bin_of[node] = b
            slot_of[node] = nslot[b]
            nslot[b] += 1
            used[b] += k
            heapq.heappush(bins, (-(rem - k), b))
    return core_of, bin_of, slot_of


def _host_prep(nodes, edges, edge_index, Wq, Wk, Wv, cfg):
    NPC, NB, NTILES, E_PAD, DVC = (
        cfg["NPC"], cfg["NB"], cfg["NTILES"], cfg["E_PAD"], cfg["DVC"])
    n_nodes, dv = nodes.shape
    nbins = NB * G

    core_of, bin_of, slot_of = _pack(edge_index, n_nodes, cfg)
    # local node index within its core
    blk = bin_of // G
    grp = bin_of % G
    loc_of = blk * P + grp * WW + slot_of            # [n_nodes]

    s = np.asarray(edge_index[0], dtype=np.int64)
    r = np.asarray(edge_index[1], dtype=np.int64)
    # order edges by (core, bin, slot) so each bin's edges are contiguous,
    # grouped receiver-by-receiver in slot order
    key = (core_of[r] * nbins + bin_of[r]) * WW + slot_of[r]
    eorder = np.argsort(key, kind="stable")
    key_s = key[eorder]
    s_s = s[eorder]
    r_s = r[eorder]

    nodes8 = np.asarray(nodes).astype(E4)
    nodes8T = np.ascontiguousarray(nodes8.T)         # [dv, n_nodes]
    edges16 = np.asarray(edges).astype(F16)

    wv16 = np.ascontiguousarray(np.asarray(Wv).T.astype(F16))        # [de, dattn]
    wk2 = np.ascontiguousarray(
        np.asarray(Wk).T.astype(E4).reshape(DVC, P, P).transpose(1, 0, 2))
    wq2 = np.ascontiguousarray(
        np.asarray(Wq).T.astype(E4).reshape(DVC, P, P).transpose(1, 0, 2))
    iota = np.tile(np.arange(WW, dtype=BF16)[None, :], (P, 1))       # [128, 32]

    in_maps = []
    meta = []
    for c in range(N_CORES):
        sel = np.searchsorted(key_s, [c * nbins * WW, (c + 1) * nbins * WW])
        ce_key = key_s[sel[0]:sel[1]]
        ce_s = s_s[sel[0]:sel[1]]
        ce_r = r_s[sel[0]:sel[1]]
        # within-bin offset for each edge
        cbin = ce_key // WW - c * nbins
        bin_start = np.searchsorted(cbin, np.arange(nbins))
        within = np.arange(cbin.size) - bin_start[cbin]
        dst = cbin * (T_SUB * P) + within
        assert within.size == 0 or (within < T_SUB * P).all()

        ebuf = np.zeros((E_PAD, P), dtype=F16)
        ebuf[dst] = edges16[eorder[sel[0]:sel[1]]]
        edgesT = np.ascontiguousarray(ebuf.T)        # [de, E_PAD]

        rc = np.full(E_PAD, 200.0, dtype=BF16)
        rc[dst] = (ce_key % WW).astype(BF16)         # slot in window
        rcolT = np.ascontiguousarray(rc.reshape(NTILES, P).T)    # [128, NTILES]

        s_arr = np.zeros(E_PAD, dtype=np.int64)
        s_arr[dst] = ce_s
        nodesT_e = np.ascontiguousarray(
            nodes8T[:, s_arr].reshape(DVC, P, E_PAD).transpose(1, 0, 2))

        # own nodes, permuted into local order
        perm = np.zeros(NPC, dtype=np.int64)         # unused slots -> node 0
        mine = core_of == c
        perm[loc_of[mine]] = np.nonzero(mine)[0]
        nodesT_own = np.ascontiguousarray(
            nodes8T[:, perm].reshape(DVC, P, NPC).transpose(1, 0, 2))

        in_maps.append(dict(
            edgesT=edgesT, rcolT=rcolT, nodesT_e=nodesT_e,
            nodesT_own=nodesT_own, wvT=wv16, wk2=wk2, wq2=wq2, iota=iota,
        ))
        meta.append((mine, loc_of))
    return in_maps, core_of, loc_of


def _pin_act_tables():
    """Restrict Bacc's activation-table choices to a single set containing
    both Exp and Lrelu, so the kernel loads the ACT table exactly once."""
    import concourse.bacc as bacc_mod
    from concourse import mybir
    if getattr(bacc_mod, "_ea_act_pinned", False):
        return
    orig = bacc_mod.get_activation_tables

    def pinned(arch):
        t = orig(arch)
        need = {mybir.ActivationFunctionType.Exp,
                mybir.ActivationFunctionType.Prelu,
                mybir.ActivationFunctionType.Relu,
                mybir.ActivationFunctionType.Copy,
                mybir.ActivationFunctionType.Identity}
        target = None
        for name, funcs in t.items():
            if need <= funcs:
                target = name
                break
        assert target is not None, "no act set with Exp+Prelu"
        return {name: (funcs if name == target else set())
                for name, funcs in t.items()}

    bacc_mod.get_activation_tables = pinned
    bacc_mod._ea_act_pinned = True


def _build_program(cfg):
    import concourse.bass as bass
    import concourse.mybir as mybir
    import concourse.tile as tile
    from concourse import bacc

    _pin_act_tables()

    f16 = mybir.dt.float16
    bf16 = mybir.dt.bfloat16
    f32 = mybir.dt.float32
    f8 = mybir.dt.float8e4
    AF = mybir.ActivationFunctionType
    ALU = mybir.AluOpType
    DR = mybir.MatmulPerfMode.DoubleRow

    NPC, NB, T_B, NTILES, E_PAD, DVC = (
        cfg["NPC"], cfg["NB"], cfg["T_B"], cfg["NTILES"], cfg["E_PAD"],
        cfg["DVC"])
    RW = P + 4                       # rhs panel stride: [v(128) | 1 | pad]
    INV_SQRT_D = 1.0 / np.sqrt(128.0)
    BNS = 8                          # subtiles per batch == per bin
    NBATCH = NTILES // BNS
    A_V = 3                          # v-act subtiles per batch done on ACT
                                     # (rest on DVE as copy + max(0.01t, t))

    nc = bacc.Bacc("TRN2", target_bir_lowering=False)
    d_edgesT = nc.dram_tensor("edgesT", [P, E_PAD], f16, kind="ExternalInput")
    d_rcolT = nc.dram_tensor("rcolT", [P, NTILES], bf16, kind="ExternalInput")
    d_nodesT_e = nc.dram_tensor(
        "nodesT_e", [P, DVC, E_PAD], f8, kind="ExternalInput")
    d_nodesT_own = nc.dram_tensor(
        "nodesT_own", [P, DVC, NPC], f8, kind="ExternalInput")
    d_wvT = nc.dram_tensor("wvT", [P, P], f16, kind="ExternalInput")
    d_wk2 = nc.dram_tensor("wk2", [P, DVC, P], f8, kind="ExternalInput")
    d_wq2 = nc.dram_tensor("wq2", [P, DVC, P], f8, kind="ExternalInput")
    d_iota = nc.dram_tensor("iota", [P, WW], bf16, kind="ExternalInput")
    d_out = nc.dram_tensor("out", [NPC, P], f32, kind="ExternalOutput")

    with tile.TileContext(nc) as tc:
        with (
            tc.tile_pool(name="persist", bufs=1) as pp,
            tc.tile_pool(name="work", bufs=5) as wk,
            tc.tile_pool(name="rhsp", bufs=5) as rp,
            tc.tile_pool(name="edma", bufs=5) as ed,
            tc.tile_pool(name="psA", bufs=2, space="PSUM") as psA,
            tc.tile_pool(name="psS", bufs=2, space="PSUM") as psS,
            tc.tile_pool(name="psO", bufs=2, space="PSUM") as psO,
        ):
            # ---- constants / persistent ----
            qT = pp.tile([P, NPC], f16, tag="qT")
            rc_all = pp.tile([P, NTILES], bf16, tag="rc")
            wvT_t = pp.tile([P, P], f16, tag="wv")
            wk2_t = pp.tile([P, DVC, P], f8, tag="wk2")
            wq2_t = pp.tile([P, DVC, P], f8, tag="wq2")
            iota_t = pp.tile([P, WW], bf16, tag="iota")
            nc.sync.dma_start(out=wvT_t[:], in_=d_wvT[:])
            nc.sync.dma_start(out=wk2_t[:], in_=d_wk2[:])
            nc.sync.dma_start(out=wq2_t[:], in_=d_wq2[:])
            nc.sync.dma_start(out=iota_t[:], in_=d_iota[:])
            nc.sync.dma_start(out=rc_all[:], in_=d_rcolT[:])

            # pre-set the ones column in every rhs-panel buffer (written
            # once; the per-batch v-act only writes cols 0..127 of panels)
            for i in range(5):
                rb = rp.tile([P, BNS, RW], bf16, tag="rhs", name=f"rhsini{i}")
                nc.gpsimd.memset(rb[:, :, P:P + 1], 1.0)

            # ---- phase 1: q chunks, interleaved into the first batches
            # (chunk i covers blocks 4i..4i+3, needed from batch 16i) ----
            def emit_q(off):
                w = min(512, NPC - off)
                qt = wk.tile([P, DVC, 512], f8, tag="qt")
                nc.sync.dma_start(
                    out=qt[:, :, :w], in_=d_nodesT_own[:, :, off:off + w])
                qps = psA.tile([P, BNS * P], f32, tag="acc")
                nc.tensor.matmul(
                    qps[:, :w], lhsT=wq2_t[:], rhs=qt[:, :, :w],
                    start=True, stop=True, perf_mode=DR)
                nc.scalar.activation(
                    out=qT[:, off:off + w], in_=qps[:, :w],
                    func=AF.Prelu, alpha=0.01)
            NQ = -(-NPC // 512)
            emit_q(0)
            emit_q(512)

            # ---- phase 2: one batch per (block, window) bin; the out
            # section of batch i-1 is emitted between batch i's front work
            # (software pipelining: hides the exp/mask latency on PE) ----
            ne = BNS * P
            state = {}                   # bi -> (Et, rhs, Et column offset)
            pend = {}
            out_ps = [None]

            def emit_front(bi):
                bt0 = bi * BNS
                bj = bt0 // T_B                      # block
                gg = (bt0 % T_B) // T_SUB            # window group
                W = gg * WW

                eT = ed.tile([P, BNS * P], f16, tag="eT")
                nc.sync.dma_start(
                    out=eT[:], in_=d_edgesT[:, bt0 * P:bt0 * P + ne])
                ntE = ed.tile([P, DVC, BNS * P], f8, tag="ntE")
                nc.sync.dma_start(
                    out=ntE[:], in_=d_nodesT_e[:, :, bt0 * P:bt0 * P + ne])

                # kT_e = lrelu(Wk @ nodes_e)  [d, e]  (DoubleRow fp8)
                kps = psA.tile([P, BNS * P], f32, tag="acc")
                for h in range(0, ne, 512):
                    nc.tensor.matmul(
                        kps[:, h:h + 512], lhsT=wk2_t[:],
                        rhs=ntE[:, :, h:h + 512],
                        start=True, stop=True, perf_mode=DR)
                kT = wk.tile([P, BNS * P], f16, tag="kT")
                nc.scalar.activation(
                    out=kT[:], in_=kps[:], func=AF.Prelu, alpha=0.01)

                # v = lrelu(edges @ Wv.T) into [v|1] panels, ACT + DVE split
                vps = psA.tile([P, BNS * P], f32, tag="acc")
                for j in range(BNS):
                    nc.tensor.matmul(
                        vps[:, j * P:(j + 1) * P],
                        lhsT=eT[:, j * P:(j + 1) * P],
                        rhs=wvT_t[:], start=True, stop=True)
                rhs = rp.tile([P, BNS, RW], bf16, tag="rhs")
                if A_V:
                    nc.scalar.activation(
                        out=rhs[:, :A_V, :P],
                        in_=vps[:, :A_V * P].rearrange(
                            "p (a n) -> p a n", n=P),
                        func=AF.Prelu, alpha=0.01)
                vtmp = wk.tile([P, (BNS - A_V) * P], bf16, tag="vtmp")
                nc.vector.tensor_scalar_mul(
                    out=vtmp[:], in0=vps[:, A_V * P:], scalar1=1.0)
                nc.vector.scalar_tensor_tensor(
                    out=rhs[:, A_V:, :P],
                    in0=vtmp[:].rearrange("p (a n) -> p a n", n=P),
                    scalar=0.01,
                    in1=vtmp[:].rearrange("p (a n) -> p a n", n=P),
                    op0=ALU.mult, op1=ALU.max)

                # S = k_e . q ; sps/exp/mask run once per batch PAIR
                half = bi % 2
                if half == 0:
                    pend["sps"] = psS.tile([P, 2 * BNS * WW], f32, tag="sps",
                                           name=f"sps{bi}")
                sps = pend["sps"]
                ho = half * BNS * WW
                for j in range(BNS):
                    nc.tensor.matmul(
                        sps[:, ho + j * WW:ho + (j + 1) * WW],
                        lhsT=kT[:, j * P:(j + 1) * P],
                        rhs=qT[:, bj * P + W:bj * P + W + WW],
                        start=True, stop=True)
                if half == 1:
                    Et = wk.tile([P, 2 * BNS * WW], bf16, tag="Et")
                    nc.scalar.activation(
                        out=Et[:], in_=sps[:], func=AF.Exp, scale=INV_SQRT_D)
                    oh = wk.tile([P, 2 * BNS * WW], bf16, tag="oh")
                    nc.vector.tensor_tensor(
                        out=oh[:].rearrange("p (a n) -> p a n", n=WW),
                        in0=rc_all[:, bt0 - BNS:bt0 + BNS, None].to_broadcast(
                            [P, 2 * BNS, WW]),
                        in1=iota_t[:, None, :].to_broadcast([P, 2 * BNS, WW]),
                        op=ALU.is_equal)
                    nc.vector.tensor_mul(out=Et[:], in0=Et[:], in1=oh[:])
                    state[bi - 1] = (Et, pend.pop("rhs0"), 0)
                    state[bi] = (Et, rhs, BNS * WW)
                else:
                    pend["rhs0"] = rhs

            def emit_back(bi):
                bt0 = bi * BNS
                bj = bt0 // T_B
                gg = (bt0 % T_B) // T_SUB
                W = gg * WW
                Et, rhs, eo = state.pop(bi)

                # out_blk[W:W+32] += Et.T @ [v | 1]  (col 128 = denom)
                if gg == 0:
                    out_ps[0] = psO.tile([P, RW], f32, tag="outp",
                                         name=f"outp{bj}")
                for j in range(BNS):
                    nc.tensor.matmul(
                        out_ps[0][W:W + WW, :P + 1],
                        lhsT=Et[:, eo + j * WW:eo + (j + 1) * WW],
                        rhs=rhs[:, j, :P + 1],
                        start=(j == 0), stop=(j == BNS - 1),
                        tile_position=(0, W),
                        skip_group_check=True)
                if gg == G - 1:
                    rec = wk.tile([P, 1], f32, tag="rec")
                    nc.vector.reciprocal(rec[:], out_ps[0][:, P:P + 1])
                    o = wk.tile([P, P], f32, tag="o")
                    nc.vector.tensor_scalar_mul(
                        out=o[:], in0=out_ps[0][:, :P], scalar1=rec[:])
                    nc.sync.dma_start(
                        out=d_out[bj * P:(bj + 1) * P, :], in_=o[:])

            for bi in range(NBATCH):
                emit_front(bi)
                if bi + 2 < NQ:
                    emit_q(512 * (bi + 2))
                if bi > 2:
                    emit_back(bi - 3)
            for bi in range(NBATCH - 3, NBATCH):
                emit_back(bi)

    nc.compile()
    return nc


def kernel(nodes, edges, edge_index, Wq, bq, Wk, bk, Wv, bv, **_unused):
    nodes = np.asarray(nodes)
    edges = np.asarray(edges)
    edge_index = np.asarray(edge_index)
    n_nodes, d_v = nodes.shape
    d_e = edges.shape[1]
    d_attn = Wq.shape[0]
    assert not np.any(bq) and not np.any(bk) and not np.any(bv), \
        "zero biases assumed"

    cfg = _cfg_from_shapes(n_nodes, d_v, d_e, d_attn)
    in_maps, core_of, loc_of = _host_prep(
        nodes, edges, edge_index,
        np.asarray(Wq), np.asarray(Wk), np.asarray(Wv), cfg)
    nc = _build_program(cfg)

    from concourse.bass_utils import run_bass_kernel_spmd
    res = run_bass_kernel_spmd(nc, in_maps, core_ids=list(range(N_CORES)))
    out = np.empty((n_nodes, d_attn), dtype=np.float32)
    for c in range(N_CORES):
        mine = np.nonzero(core_of == c)[0]
        out[mine] = res.results[c]["out"][loc_of[mine]]
    # receivers with no incident edges produce 0 rows in the reference
    cnt = np.bincount(np.asarray(edge_index[1], dtype=np.int64),
                      minlength=n_nodes)
    out[cnt == 0] = 0.0
    return np.ascontiguousarray(out)


# revision 33
# speedup vs baseline: 1.2126x; 1.2126x over previous
"""EdgeAttention GNN message passing on 8 Trainium2 NeuronCores.

Strategy (edge-parallel, receiver-packed, gather-free, fp8 senders):
  - Host: receivers are bin-packed into (core, block, window) bins --
    8 cores x 50 blocks x 4 windows of 32 receiver slots, each bin
    holding at most t=8 subtiles (1024 edge slots).  Receiver->slot
    assignment is a free permutation (host unpermutes the output), so
    edge counts balance almost perfectly and padding is ~2%.  The host
    materializes each edge slot's SENDER raw features (pure data
    movement) in fp8-e4m3 DoubleRow layout; edge features stay f16
    (the v-path error budget does not tolerate fp8; the k/q path does).
  - Device phase 1: q = lrelu(own_nodes @ Wq.T) via one fp8 DoubleRow
    matmul per 512 columns, kept in SBUF (f16).
  - Device phase 2, one batch == one (block, window) bin == 8 subtiles:
      kT_e = lrelu(Wk2 @r nodesT_e)   (fp8 DoubleRow PE + ACT Prelu)
      v    = lrelu(edgesT @ Wv.T)     (f16 PE per subtile; lrelu split
                                       ACT Prelu / DVE copy+max(t,.01t))
      S    = kT_e.T @ q_win           (f16 PE, 32 cols per subtile)
      Et   = exp(S / sqrt(d))         (ACT, one instr per batch)
      oh   = is_equal(rc, iota32)     (DVE); Et *= oh
      out_blk[W:W+32] += Et.T @ [v|1] (PE; col 128 = softmax denom;
                                       emitted one batch late to hide
                                       the exp/mask latency)
    Block epilogue: out = numer * reciprocal(denom) (DVE), DMA out.
  Softmax max-subtraction is skipped: logits are O(1) here (|l|<4) and
  exp(l)/sum(exp(l)) == exp(l-m)/sum(exp(l-m)) exactly in the reals.
"""

import sys

sys.path.insert(0, "/opt/trn_rl_repo")

import numpy as np
import ml_dtypes

N_CORES = 8
P = 128
WW = 32                              # receiver window width (slots per bin)
G = P // WW                          # windows per block
T_SUB = 8                            # subtiles per bin (bin edge cap T_SUB*P)

E4 = ml_dtypes.float8_e4m3
F16 = np.float16
BF16 = ml_dtypes.bfloat16


def _cfg_from_shapes(n_nodes, d_v, d_e, d_attn):
    assert d_v == 2 * P and d_e == P and d_attn == P
    npc = -(-n_nodes // (N_CORES * P)) * P + P      # one spare block of slack
    nb = npc // P
    t_b = G * T_SUB
    ntiles = nb * t_b
    return dict(
        NPC=npc, NB=nb, T_B=t_b, NTILES=ntiles, E_PAD=ntiles * P, DVC=d_v // P,
    )


def _pack(edge_index, n_nodes, cfg):
    """Assign receivers to (core, bin, slot); bins are (block, window).

    Returns core_of[r], bin_of[r], slot_of[r]."""
    import heapq

    NB = cfg["NB"]
    nbins = NB * G
    cap = T_SUB * P
    r = np.asarray(edge_index[1], dtype=np.int64)
    cnt = np.bincount(r, minlength=n_nodes)
    order = np.argsort(-cnt, kind="stable")

    core_of = np.full(n_nodes, -1, dtype=np.int64)
    bin_of = np.full(n_nodes, -1, dtype=np.int64)
    slot_of = np.full(n_nodes, -1, dtype=np.int64)

    # 1) balance edges across cores (receiver-count cap NPC slots)
    core_heap = [(0, c) for c in range(N_CORES)]
    heapq.heapify(core_heap)
    core_recv = [[] for _ in range(N_CORES)]
    max_recv = nbins * WW
    for node in order:
        while True:
            load, c = heapq.heappop(core_heap)
            if len(core_recv[c]) < max_recv:
                break
        core_recv[c].append(node)
        core_of[node] = c
        heapq.heappush(core_heap, (load + int(cnt[node]), c))

    # 2) per core: worst-fit-decreasing into bins (cap edges, 32 slots)
    for c in range(N_CORES):
        bins = [(-cap, b) for b in range(nbins)]
        heapq.heapify(bins)
        used = np.zeros(nbins, dtype=np.int64)
        nslot = np.zeros(nbins, dtype=np.int64)
        for node in core_recv[c]:          # already count-descending
            while True:
                negrem, b = heapq.heappop(bins)
                if nslot[b] < WW:
                    break
            rem = -negrem
            k = int(cnt[node])
            assert rem >= k, "bin packing overflow; raise T_SUB"
            bin_of[node] = b
            slot_of[node] = nslot[b]
            nslot[b] += 1
            used[b] += k
            heapq.heappush(bins, (-(rem - k), b))
    return core_of, bin_of, slot_of


def _host_prep(nodes, edges, edge_index, Wq, Wk, Wv, cfg):
    NPC, NB, NTILES, E_PAD, DVC = (
        cfg["NPC"], cfg["NB"], cfg["NTILES"], cfg["E_PAD"], cfg["DVC"])
    n_nodes, dv = nodes.shape
    nbins = NB * G

    core_of, bin_of, slot_of = _pack(edge_index, n_nodes, cfg)
    # local node index within its core
    blk = bin_of // G
    grp = bin_of % G
    loc_of = blk * P + grp * WW + slot_of            # [n_nodes]

    s = np.asarray(edge_index[0], dtype=np.int64)
    r = np.asarray(edge_index[1], dtype=np.int64)
    # order edges by (core, bin, slot) so each bin's edges are contiguous,
    # grouped receiver-by-receiver in slot order
    key = (core_of[r] * nbins + bin_of[r]) * WW + slot_of[r]
    eorder = np.argsort(key, kind="stable")
    key_s = key[eorder]
    s_s = s[eorder]
    r_s = r[eorder]

    nodes8 = np.asarray(nodes).astype(E4)
    nodes8T = np.ascontiguousarray(nodes8.T)         # [dv, n_nodes]
    edges16 = np.asarray(edges).astype(F16)

    wv16 = np.ascontiguousarray(np.asarray(Wv).T.astype(F16))        # [de, dattn]
    wk2 = np.ascontiguousarray(
        np.asarray(Wk).T.astype(E4).reshape(DVC, P, P).transpose(1, 0, 2))
    wq2 = np.ascontiguousarray(
        np.asarray(Wq).T.astype(E4).reshape(DVC, P, P).transpose(1, 0, 2))
    iota = np.tile(np.arange(WW, dtype=BF16)[None, :], (P, 1))       # [128, 32]

    in_maps = []
    meta = []
    for c in range(N_CORES):
        sel = np.searchsorted(key_s, [c * nbins * WW, (c + 1) * nbins * WW])
        ce_key = key_s[sel[0]:sel[1]]
        ce_s = s_s[sel[0]:sel[1]]
        ce_r = r_s[sel[0]:sel[1]]
        # within-bin offset for each edge
        cbin = ce_key // WW - c * nbins
        bin_start = np.searchsorted(cbin, np.arange(nbins))
        within = np.arange(cbin.size) - bin_start[cbin]
        dst = cbin * (T_SUB * P) + within
        assert within.size == 0 or (within < T_SUB * P).all()

        ebuf = np.zeros((E_PAD, P), dtype=F16)
        ebuf[dst] = edges16[eorder[sel[0]:sel[1]]]
        edgesT = np.ascontiguousarray(ebuf.T)        # [de, E_PAD]

        rc = np.full(E_PAD, 200.0, dtype=BF16)
        rc[dst] = (ce_key % WW).astype(BF16)         # slot in window
        rcolT = np.ascontiguousarray(rc.reshape(NTILES, P).T)    # [128, NTILES]

        s_arr = np.zeros(E_PAD, dtype=np.int64)
        s_arr[dst] = ce_s
        nodesT_e = np.ascontiguousarray(
            nodes8T[:, s_arr].reshape(DVC, P, E_PAD).transpose(1, 0, 2))

        # own nodes, permuted into local order
        perm = np.zeros(NPC, dtype=np.int64)         # unused slots -> node 0
        mine = core_of == c
        perm[loc_of[mine]] = np.nonzero(mine)[0]
        nodesT_own = np.ascontiguousarray(
            nodes8T[:, perm].reshape(DVC, P, NPC).transpose(1, 0, 2))

        in_maps.append(dict(
            edgesT=edgesT, rcolT=rcolT, nodesT_e=nodesT_e,
            nodesT_own=nodesT_own, wvT=wv16, wk2=wk2, wq2=wq2, iota=iota,
        ))
        meta.append((mine, loc_of))
    return in_maps, core_of, loc_of


def _pin_act_tables():
    """Restrict Bacc's activation-table choices to a single set containing
    both Exp and Lrelu, so the kernel loads the ACT table exactly once."""
    import concourse.bacc as bacc_mod
    from concourse import mybir
    if getattr(bacc_mod, "_ea_act_pinned", False):
        return
    orig = bacc_mod.get_activation_tables

    def pinned(arch):
        t = orig(arch)
        need = {mybir.ActivationFunctionType.Exp,
                mybir.ActivationFunctionType.Prelu,
                mybir.ActivationFunctionType.Relu,
                mybir.ActivationFunctionType.Copy,
                mybir.ActivationFunctionType.Identity}
        target = None
        for name, funcs in t.items():
            if need <= funcs:
                target = name
                break
        assert target is not None, "no act set with Exp+Prelu"
        return {name: (funcs if name == target else set())
                for name, funcs in t.items()}

    bacc_mod.get_activation_tables = pinned
    bacc_mod._ea_act_pinned = True


def _build_program(cfg):
    import concourse.bass as bass
    import concourse.mybir as mybir
    import concourse.tile as tile
    from concourse import bacc

    _pin_act_tables()

    f16 = mybir.dt.float16
    bf16 = mybir.dt.bfloat16
    f32 = mybir.dt.float32
    f8 = mybir.dt.float8e4
    AF = mybir.ActivationFunctionType
    ALU = mybir.AluOpType
    DR = mybir.MatmulPerfMode.DoubleRow

    NPC, NB, T_B, NTILES, E_PAD, DVC = (
        cfg["NPC"], cfg["NB"], cfg["T_B"], cfg["NTILES"], cfg["E_PAD"],
        cfg["DVC"])
    RW = P + 4                       # rhs panel stride: [v(128) | 1 | pad]
    INV_SQRT_D = 1.0 / np.sqrt(128.0)
    BNS = 8                          # subtiles per batch == per bin
    NBATCH = NTILES // BNS
    A_V = 3                          # v-act subtiles per batch done on ACT
                                     # (rest on DVE as copy + max(0.01t, t))

    nc = bacc.Bacc("TRN2", target_bir_lowering=False)
    d_edgesT = nc.dram_tensor("edgesT", [P, E_PAD], f16, kind="ExternalInput")
    d_rcolT = nc.dram_tensor("rcolT", [P, NTILES], bf16, kind="ExternalInput")
    d_nodesT_e = nc.dram_tensor(
        "nodesT_e", [P, DVC, E_PAD], f8, kind="ExternalInput")
    d_nodesT_own = nc.dram_tensor(
        "nodesT_own", [P, DVC, NPC], f8, kind="ExternalInput")
    d_wvT = nc.dram_tensor("wvT", [P, P], f16, kind="ExternalInput")
    d_wk2 = nc.dram_tensor("wk2", [P, DVC, P], f8, kind="ExternalInput")
    d_wq2 = nc.dram_tensor("wq2", [P, DVC, P], f8, kind="ExternalInput")
    d_iota = nc.dram_tensor("iota", [P, WW], bf16, kind="ExternalInput")
    d_out = nc.dram_tensor("out", [NPC, P], f32, kind="ExternalOutput")

    with tile.TileContext(nc) as tc:
        with (
            tc.tile_pool(name="persist", bufs=1) as pp,
            tc.tile_pool(name="work", bufs=5) as wk,
            tc.tile_pool(name="rhsp", bufs=5) as rp,
            tc.tile_pool(name="edma", bufs=5) as ed,
            tc.tile_pool(name="psA", bufs=2, space="PSUM") as psA,
            tc.tile_pool(name="psS", bufs=2, space="PSUM") as psS,
            tc.tile_pool(name="psO", bufs=2, space="PSUM") as psO,
        ):
            # ---- constants / persistent ----
            qT = pp.tile([P, NPC], f16, tag="qT")
            rc_all = pp.tile([P, NTILES], bf16, tag="rc")
            wvT_t = pp.tile([P, P], f16, tag="wv")
            wk2_t = pp.tile([P, DVC, P], f8, tag="wk2")
            wq2_t = pp.tile([P, DVC, P], f8, tag="wq2")
            iota_t = pp.tile([P, WW], bf16, tag="iota")
            nc.sync.dma_start(out=wvT_t[:], in_=d_wvT[:])
            nc.sync.dma_start(out=wk2_t[:], in_=d_wk2[:])
            nc.sync.dma_start(out=wq2_t[:], in_=d_wq2[:])
            nc.sync.dma_start(out=iota_t[:], in_=d_iota[:])
            nc.sync.dma_start(out=rc_all[:], in_=d_rcolT[:])

            # pre-set the ones column in every rhs-panel buffer (written
            # once; the per-batch v-act only writes cols 0..127 of panels)
            for i in range(5):
                rb = rp.tile([P, BNS, RW], bf16, tag="rhs", name=f"rhsini{i}")
                nc.gpsimd.memset(rb[:, :, P:P + 1], 1.0)

            # ---- phase 1: q for own nodes (fp8 DoubleRow) ----
            off = 0
            while off < NPC:
                w = min(512, NPC - off)
                qt = wk.tile([P, DVC, 512], f8, tag="qt")
                nc.sync.dma_start(
                    out=qt[:, :, :w], in_=d_nodesT_own[:, :, off:off + w])
                qps = psA.tile([P, BNS * P], f32, tag="acc")
                nc.tensor.matmul(
                    qps[:, :w], lhsT=wq2_t[:], rhs=qt[:, :, :w],
                    start=True, stop=True, perf_mode=DR)
                nc.scalar.activation(
                    out=qT[:, off:off + w], in_=qps[:, :w],
                    func=AF.Prelu, alpha=0.01)
                off += w

            # ---- phase 2: one batch per (block, window) bin; the out
            # section of batch i-1 is emitted between batch i's front work
            # (software pipelining: hides the exp/mask latency on PE) ----
            ne = BNS * P
            state = {}                   # bi -> (Et, rhs)
            out_ps = [None]

            def emit_front(bi):
                bt0 = bi * BNS
                bj = bt0 // T_B                      # block
                gg = (bt0 % T_B) // T_SUB            # window group
                W = gg * WW

                eT = ed.tile([P, BNS * P], f16, tag="eT")
                nc.sync.dma_start(
                    out=eT[:], in_=d_edgesT[:, bt0 * P:bt0 * P + ne])
                ntE = ed.tile([P, DVC, BNS * P], f8, tag="ntE")
                nc.sync.dma_start(
                    out=ntE[:], in_=d_nodesT_e[:, :, bt0 * P:bt0 * P + ne])

                # kT_e = lrelu(Wk @ nodes_e)  [d, e]  (DoubleRow fp8)
                kps = psA.tile([P, BNS * P], f32, tag="acc")
                for h in range(0, ne, 512):
                    nc.tensor.matmul(
                        kps[:, h:h + 512], lhsT=wk2_t[:],
                        rhs=ntE[:, :, h:h + 512],
                        start=True, stop=True, perf_mode=DR)
                kT = wk.tile([P, BNS * P], f16, tag="kT")
                nc.scalar.activation(
                    out=kT[:], in_=kps[:], func=AF.Prelu, alpha=0.01)

                # v = lrelu(edges @ Wv.T) into [v|1] panels, ACT + DVE split
                vps = psA.tile([P, BNS * P], f32, tag="acc")
                for j in range(BNS):
                    nc.tensor.matmul(
                        vps[:, j * P:(j + 1) * P],
                        lhsT=eT[:, j * P:(j + 1) * P],
                        rhs=wvT_t[:], start=True, stop=True)
                rhs = rp.tile([P, BNS, RW], bf16, tag="rhs")
                if A_V:
                    nc.scalar.activation(
                        out=rhs[:, :A_V, :P],
                        in_=vps[:, :A_V * P].rearrange(
                            "p (a n) -> p a n", n=P),
                        func=AF.Prelu, alpha=0.01)
                vtmp = wk.tile([P, (BNS - A_V) * P], bf16, tag="vtmp")
                nc.vector.tensor_scalar_mul(
                    out=vtmp[:], in0=vps[:, A_V * P:], scalar1=1.0)
                nc.vector.scalar_tensor_tensor(
                    out=rhs[:, A_V:, :P],
                    in0=vtmp[:].rearrange("p (a n) -> p a n", n=P),
                    scalar=0.01,
                    in1=vtmp[:].rearrange("p (a n) -> p a n", n=P),
                    op0=ALU.mult, op1=ALU.max)

                # S = k_e . q over the bin's 32-receiver window
                sps = psS.tile([P, BNS * WW], f32, tag="sps")
                for j in range(BNS):
                    nc.tensor.matmul(
                        sps[:, j * WW:(j + 1) * WW],
                        lhsT=kT[:, j * P:(j + 1) * P],
                        rhs=qT[:, bj * P + W:bj * P + W + WW],
                        start=True, stop=True)
                Et = wk.tile([P, BNS * WW], bf16, tag="Et")
                nc.scalar.activation(
                    out=Et[:], in_=sps[:], func=AF.Exp, scale=INV_SQRT_D)

                # mask: oh[e, w] = (rc[e] == iota[w]); Et *= oh
                oh = wk.tile([P, BNS * WW], bf16, tag="oh")
                nc.vector.tensor_tensor(
                    out=oh[:].rearrange("p (a n) -> p a n", n=WW),
                    in0=rc_all[:, bt0:bt0 + BNS, None].to_broadcast(
                        [P, BNS, WW]),
                    in1=iota_t[:, None, :].to_broadcast([P, BNS, WW]),
                    op=ALU.is_equal)
                nc.vector.tensor_mul(out=Et[:], in0=Et[:], in1=oh[:])
                state[bi] = (Et, rhs)

            def emit_back(bi):
                bt0 = bi * BNS
                bj = bt0 // T_B
                gg = (bt0 % T_B) // T_SUB
                W = gg * WW
                Et, rhs = state.pop(bi)

                # out_blk[W:W+32] += Et.T @ [v | 1]  (col 128 = denom)
                if gg == 0:
                    out_ps[0] = psO.tile([P, RW], f32, tag="outp",
                                         name=f"outp{bj}")
                for j in range(BNS):
                    nc.tensor.matmul(
                        out_ps[0][W:W + WW, :P + 1],
                        lhsT=Et[:, j * WW:(j + 1) * WW],
                        rhs=rhs[:, j, :P + 1],
                        start=(j == 0), stop=(j == BNS - 1),
                        tile_position=(0, W),
                        skip_group_check=True)
                if gg == G - 1:
                    rec = wk.tile([P, 1], f32, tag="rec")
                    nc.vector.reciprocal(rec[:], out_ps[0][:, P:P + 1])
                    o = wk.tile([P, P], f32, tag="o")
                    nc.vector.tensor_scalar_mul(
                        out=o[:], in0=out_ps[0][:, :P], scalar1=rec[:])
                    nc.sync.dma_start(
                        out=d_out[bj * P:(bj + 1) * P, :], in_=o[:])

            for bi in range(NBATCH):
                emit_front(bi)
                if bi > 2:
                    emit_back(bi - 3)
            for bi in range(NBATCH - 3, NBATCH):
                emit_back(bi)

    nc.compile()
    return nc


def kernel(nodes, edges, edge_index, Wq, bq, Wk, bk, Wv, bv, **_unused):
    nodes = np.asarray(nodes)
    edges = np.asarray(edges)
    edge_index = np.asarray(edge_index)
    n_nodes, d_v = nodes.shape
    d_e = edges.shape[1]
    d_attn = Wq.shape[0]
    assert not np.any(bq) and not np.any(bk) and not np.any(bv), \
        "zero biases assumed"

    cfg = _cfg_from_shapes(n_nodes, d_v, d_e, d_attn)
    in_maps, core_of, loc_of = _host_prep(
        nodes, edges, edge_index,
        np.asarray(Wq), np.asarray(Wk), np.asarray(Wv), cfg)
    nc = _build_program(cfg)

    from concourse.bass_utils import run_bass_kernel_spmd
    res = run_bass_kernel_spmd(nc, in_maps, core_ids=list(range(N_CORES)))
    out = np.empty((n_nodes, d_attn), dtype=np.float32)
    for c in range(N_CORES):
        mine = np.nonzero(core_of == c)[0]
        out[mine] = res.results[c]["out"][loc_of[mine]]
    # receivers with no incident edges produce 0 rows in the reference
    cnt = np.bincount(np.asarray(edge_index[1], dtype=np.int64),
                      minlength=n_nodes)
    out[cnt == 0] = 0.0
    return np.ascontiguousarray(out)
